# revision 21
# baseline (speedup 1.0000x reference)
"""Causal multi-head attention block on 8 TRN2 NeuronCores.

Sharding: tensor-parallel over heads (16 heads -> 2 per core) for QKV +
attention; AllToAll switches to token-parallel (4096 tokens -> 512 per
core) for the output projection.

Per-core device graph (all bf16 matmuls, fp32 PSUM accumulation):
  1. QKV: q^T,k^T in [e,t] layout (e = 2 heads x 64 on partitions),
     v in [t,e] layout, from a resident x^T [1024, 4096].
  2. Attention per (batch, head): scores computed TRANSPOSED
     s^T[kv, q] = k^T.T @ q^T so softmax stats land on the free axis of
     nothing -- instead the denominator comes free from a ones-column
     appended to v (rows of attn^T psum: 0:64 = numerator, 64 = denom).
     No max-subtraction (scores ~ N(0,1) after folding 1/sqrt(dk) into
     w_q on the host; exp can't overflow).
  3. Normalize via reciprocal + gpsimd partition-broadcast, stage to a
     DRAM AllToAll buffer as bf16.
  4. AllToAll (head-shards -> token-shards), then out = attn^T.T @ w_p^T
     per 512-token chunk, + host-folded constant bias vector.
"""

import numpy as np
import ml_dtypes

import concourse.bass as bass
import concourse.bacc as bacc
import concourse.mybir as mybir
from concourse.tile import TileContext, add_dep_helper
from concourse.bass_utils import run_bass_kernel_spmd

NC = 8                      # cores
B, S, D = 2, 2048, 1024
H, DK = 16, 64
HPC = H // NC               # heads per core = 2
EC = HPC * DK               # embed dims per core = 128
T = B * S                   # 4096 flattened tokens
TC = T // NC                # tokens per core chunk = 512
K8 = D // 128               # contraction tiles = 8
SCALE = 1.0 / np.sqrt(DK)

BF16 = mybir.dt.bfloat16
F32 = mybir.dt.float32
NPBF16 = ml_dtypes.bfloat16

_CACHE = {}


def _build_nc(dbg: bool = False) -> bass.Bass:
    nc = bacc.Bacc("TRN2", target_bir_lowering=False, debug=False, num_devices=NC)
    if dbg:
        dbg_q = nc.declare_dram_parameter("dbg_q", [EC, T], BF16, isOutput=True)
        dbg_k = nc.declare_dram_parameter("dbg_k", [EC, T], BF16, isOutput=True)
        dbg_v = nc.declare_dram_parameter("dbg_v", [128, T // 128 * 130], BF16, isOutput=True)
        dbg_ain = nc.declare_dram_parameter("dbg_ain", [NC, 128, 512], BF16, isOutput=True)
        dbg_aout = nc.declare_dram_parameter("dbg_aout", [NC, 128, 512], BF16, isOutput=True)

    xT = nc.declare_dram_parameter("xT", [D, T], BF16, isOutput=False)
    wqkvT = nc.declare_dram_parameter("wqkvT", [D, 3 * EC], BF16, isOutput=False)
    wpT = nc.declare_dram_parameter("wpT", [D, D], BF16, isOutput=False)
    cvec = nc.declare_dram_parameter("cvec", [1, D], F32, isOutput=False)
    out = nc.declare_dram_parameter("out", [TC, D], F32, isOutput=True)

    # causal masking happens on the PE: a matmul accumulates
    # ident.T @ maskneg = -60 on the strict lower triangle (kv > q) into
    # the diagonal score tiles, so exp() itself produces ~0 there and no
    # vector-engine op (and no extra ACT sync wait) is needed.
    ident_np = np.eye(128, dtype=NPBF16)
    maskneg_np = np.where(np.arange(128)[:, None] > np.arange(128)[None, :],
                          -60.0, 0.0).astype(NPBF16)
    ident_dram = nc.inline_tensor(ident_np, name="ident128")
    maskneg_dram = nc.inline_tensor(maskneg_np, name="maskneg128")

    with TileContext(nc) as tc:
        with (
            tc.tile_pool(name="const", bufs=1) as constp,
            tc.tile_pool(name="x", bufs=1) as xp,
            tc.tile_pool(name="qk", bufs=1) as qkp,
            tc.tile_pool(name="w", bufs=1) as wp,
            tc.tile_pool(name="ps", bufs=8, space="PSUM") as psp,
            tc.tile_pool(name="pt", bufs=6) as ptp,
            tc.tile_pool(name="nrm", bufs=2) as nrmp,
            tc.tile_pool(name="stage", bufs=4) as stp,
            tc.tile_pool(name="dram", bufs=1, space="DRAM") as dramp,
            tc.tile_pool(name="proj", bufs=1) as projp,
        ):
            # ---- constants ----
            # DMA-loaded tiles that feed DVE ops get "pre-touched" by a DVE
            # copy: the DVE clock absorbs the DMA-queue wait once, so the hot
            # consumers carry only their PE/ACT wait (the tensor_scalar ISA
            # struct can't encode two sync waits).
            ident_sb = constp.tile([128, 128], BF16)
            nc.sync.dma_start(out=ident_sb[:, :], in_=ident_dram[:, :])
            maskneg_sb = constp.tile([128, 128], BF16)
            nc.sync.dma_start(out=maskneg_sb[:, :], in_=maskneg_dram[:, :])
            cv_ld = constp.tile([128, D], F32)
            nc.gpsimd.dma_start(out=cv_ld[:, :], in_=cvec[:, :].to_broadcast([128, D]))
            cv_b = constp.tile([128, D], F32)
            nc.vector.tensor_copy(cv_b[:, :], cv_ld[:, :])

            # ---- load x^T and weights (interleaved so the first QKV
            # matmul's inputs land early; w_proj loads late) ----
            x_sb = xp.tile([128, K8, T], BF16)          # 8 MB
            wqkv_sb = wp.tile([128, K8, 3 * EC], BF16)
            for k in range(K8):
                nc.sync.dma_start(
                    out=wqkv_sb[:, k, :], in_=wqkvT[k * 128:(k + 1) * 128, :]
                )
                nc.sync.dma_start(out=x_sb[:, k, 0:T // 2],
                                  in_=xT[k * 128:(k + 1) * 128, 0:T // 2])
                nc.sync.dma_start(out=x_sb[:, k, T // 2:T],
                                  in_=xT[k * 128:(k + 1) * 128, T // 2:T])
            wp_sb = wp.tile([128, K8, D], BF16)         # w_proj^T (loaded late)

            q_sb = qkp.tile([EC, T], BF16)
            k_sb = qkp.tile([EC, T], BF16)
            # v layout: per 128-token tile, [v_h0(64) | ones | v_h1(64) | ones]
            v_sb = qkp.tile([128, T // 128, 130], BF16)
            nc.vector.memset(v_sb[:, :, 64:65], 1.0)    # ones column, head 0
            nc.vector.memset(v_sb[:, :, 129:130], 1.0)  # ones column, head 1

            # PSUM tag budget (8 banks): pso 3 + pss 2x2 + fill 1 = 8
            def qkv_stream(b):
                """Emit batch b's QKV, one instruction per next().

                q/k: two 512-token groups per 2-bank "pss" slot. v: four
                128-token tiles per slot sharing one bank -- only the very
                first matmul of the slot uses start=True (start clears the
                whole bank; zeroed regions accumulate correctly).
                """
                base = b * S
                for sec in (0, 1):
                    dst = q_sb if sec == 0 else k_sb
                    for np_ in range(2):                # pair of 512-groups
                        pq = psp.tile([128, 2, 512], F32, tag="pss", bufs=2,
                                      name=f"fq{b}{sec}{np_}")
                        for k in range(K8):
                            for n2 in range(2):
                                nc.tensor.matmul(
                                    pq[:, n2, :],
                                    lhsT=wqkv_sb[:, k, sec * EC:(sec + 1) * EC],
                                    rhs=x_sb[:, k, base + (np_ * 2 + n2) * 512:
                                             base + (np_ * 2 + n2 + 1) * 512],
                                    start=(k == 0), stop=(k == K8 - 1),
                                )
                                yield
                        nc.vector.tensor_copy(
                            dst[:, base + np_ * 1024:base + (np_ + 1) * 1024],
                            pq[:, :, :].rearrange("p a b -> p (a b)"),
                        )
                        yield
                for tq in range(4):                     # quad of v token-tiles
                    pv = psp.tile([128, 4, 128], F32, tag="pss", bufs=2,
                                  name=f"fv{b}{tq}")
                    for k in range(K8):
                        for t2 in range(4):
                            nc.tensor.matmul(
                                pv[:, t2, :],
                                lhsT=x_sb[:, k, base + (tq * 4 + t2) * 128:
                                          base + (tq * 4 + t2 + 1) * 128],
                                rhs=wqkv_sb[:, k, 2 * EC:3 * EC],
                                start=(k == 0 and t2 == 0), stop=(k == K8 - 1),
                            )
                            yield
                    src4 = pv[:, :, :].rearrange("p t (h e) -> p t h e", h=2)
                    dst4 = v_sb[:, b * 16 + tq * 4:b * 16 + (tq + 1) * 4, :]\
                        .rearrange("p t (h e) -> p t h e", e=65)[:, :, :, 0:64]
                    nc.vector.tensor_copy(dst4, src4)
                    yield

            last_dummy = [None]

            def dummy_stream():
                """Endless PE keep-warm matmuls into a scratch PSUM bank."""
                pd = psp.tile([128, 512], F32, tag="fill", bufs=1, name="dummy_ps")
                while True:
                    last_dummy[0] = nc.tensor.matmul(
                        pd[:, :], lhsT=wqkv_sb[:, 0, 0:EC],
                        rhs=x_sb[:, 0, 0:512], start=True, stop=True,
                    )
                    yield

            def drain(it, n):
                for _ in range(n):
                    try:
                        next(it)
                    except StopIteration:
                        return

            a2a_in = dramp.tile([NC, 128, 512], BF16, name="a2a_in")
            dummies = dummy_stream()

            def emit_attention(b, fill, fpi):
                """Attention for batch b; `fill` instructions are woven in at
                `fpi` per kv-tile iteration to keep the PE dense while ACT
                runs exp. Both heads share one 2-bank score tile so exp is a
                single ACT instruction per kv-tile."""
                gorder = range(S // 512) if b == 0 else \
                    reversed(range(S // 512))
                for g in gorder:                        # 4 query groups
                    pso = [psp.tile([128, 512], F32, tag="pso", bufs=3,
                                    name=f"pso_{b}_{g}_{hi}") for hi in range(2)]
                    nkv = 4 * (g + 1)
                    prev = None
                    for kj in range(nkv):
                        d = kj - 4 * g                  # >=0 on diagonal band
                        j0 = 128 * d if d >= 0 else 0
                        pss2 = psp.tile([128, 2, 512], F32, tag="pss", bufs=2,
                                        name=f"pss_{b}_{g}_{kj}")
                        for hi in range(2):
                            h0 = hi * DK
                            nc.tensor.matmul(
                                pss2[:, hi, j0:512],
                                lhsT=k_sb[h0:h0 + DK,
                                          b * S + kj * 128:b * S + (kj + 1) * 128],
                                rhs=q_sb[h0:h0 + DK,
                                         b * S + g * 512 + j0:b * S + (g + 1) * 512],
                                start=True, stop=(d < 0),
                            )
                            if d >= 0:
                                nc.tensor.matmul(
                                    pss2[:, hi, j0:j0 + 128],
                                    lhsT=ident_sb[:, :], rhs=maskneg_sb[:, :],
                                    start=False, stop=True,
                                )
                        pt2 = ptp.tile([128, 2, 512], BF16, tag="pt",
                                       name=f"pt_{b}_{g}_{kj}")
                        nc.scalar.activation(
                            pt2[:, :, j0:512], pss2[:, :, j0:512],
                            mybir.ActivationFunctionType.Exp,
                        )
                        if prev is not None:
                            ppt, pj0, pkj = prev
                            for hi in range(2):
                                nc.tensor.matmul(
                                    pso[hi][:65, pj0:512],
                                    lhsT=v_sb[:, (b * S) // 128 + pkj,
                                              hi * 65:(hi + 1) * 65],
                                    rhs=ppt[:, hi, pj0:512],
                                    start=(pkj == 0), stop=False,
                                )
                        drain(fill, fpi)
                        prev = (pt2, j0, kj)
                    ppt, pj0, pkj = prev
                    for hi in range(2):
                        nc.tensor.matmul(
                            pso[hi][:65, pj0:512],
                            lhsT=v_sb[:, (b * S) // 128 + pkj,
                                      hi * 65:(hi + 1) * 65],
                            rhs=ppt[:, hi, pj0:512],
                            start=(pkj == 0), stop=True,
                        )
                    # normalize rows 0:64 by row 64, stage for A2A
                    for hi in range(2):
                        dn = nrmp.tile([1, 512], F32, tag="dn")
                        nc.vector.tensor_copy(dn[:, :], pso[hi][64:65, :])
                        ddr = dramp.tile([1, 512], F32, tag="ddr", bufs=4,
                                         name="ddr")
                        nc.sync.dma_start(out=ddr[:, :], in_=dn[:, :])
                        dnb = nrmp.tile([64, 512], F32, tag="dnb")
                        nc.gpsimd.dma_start(
                            out=dnb[:, :], in_=ddr[:, :].to_broadcast([64, 512])
                        )
                        rb = nrmp.tile([64, 512], F32, tag="rb")
                        nc.vector.reciprocal_approx_fast(rb[:, :], dnb[:, :])
                        aout = stp.tile([64, 512], BF16, tag="aout", bufs=3)
                        nc.vector.tensor_mul(aout[:, :], pso[hi][0:64, :], rb[:, :])
                        chunk = b * (S // 512) + g
                        nc.sync.dma_start(
                            out=a2a_in[chunk, hi * 64:(hi + 1) * 64, :],
                            in_=aout[:, :],
                        )

            # batch 0 QKV runs dense; batch 1 QKV weaves into batch 0's
            # attention; dummies keep HAM warm through batch 1's attention.
            qkv0 = qkv_stream(0)
            drain(qkv0, 10 ** 6)
            fill1 = qkv_stream(1)
            emit_attention(0, fill1, 3)
            drain(fill1, 10 ** 6)
            emit_attention(1, dummies, 1)

            if dbg:
                nc.sync.dma_start(out=dbg_q[:, :], in_=q_sb[:, :])
                nc.sync.dma_start(out=dbg_k[:, :], in_=k_sb[:, :])
                nc.sync.dma_start(out=dbg_v[:, :], in_=v_sb[:, :, :].rearrange("p a b -> p (a b)"))
                nc.sync.dma_start(out=dbg_ain[:, :, :], in_=a2a_in[:, :, :])
            # w_proj loads during attention; A2A bridge with PE kept warm
            for k in range(K8):
                nc.sync.dma_start(out=wp_sb[:, k, :], in_=wpT[k * 128:(k + 1) * 128, :])
            a2a_out = dramp.tile([NC, 128, 512], BF16, name="a2a_out")
            nc.gpsimd.collective_compute(
                "AllToAll",
                mybir.AluOpType.bypass,
                ins=[a2a_in.opt()],
                outs=[a2a_out.opt()],
                replica_groups=[list(range(NC))],
            )
            drain(dummies, 128)

            if dbg:
                nc.sync.dma_start(out=dbg_aout[:, :, :], in_=a2a_out[:, :, :])
            # ---- output projection on my 512-token chunk ----
            at_sb = projp.tile([128, NC, 512], BF16)
            for k in range(NC):
                nc.sync.dma_start(out=at_sb[:, k, :], in_=a2a_out[k, :, :])
            # redirect proj reads through DVE copies: the matmuls then wait
            # only on the DVE semaphore (one wait -> no LDWEIGHTS hoisting,
            # which otherwise stalls the PE queue on the A2A)
            at2 = projp.tile([128, NC, 512], BF16)
            nc.vector.tensor_copy(at2[:, :, :], at_sb[:, :, :])
            wp2 = projp.tile([128, K8, D], BF16)
            nc.vector.tensor_copy(wp2[:, :, :], wp_sb[:, :, :])
            for ti in range(TC // 128):
                pspj = psp.tile([128, 2, 512], F32, tag="pss", bufs=2,
                                name=f"pspj_{ti}")
                for ng in range(2):
                    for k in range(K8):
                        mm = nc.tensor.matmul(
                            pspj[:, ng, :],
                            lhsT=at2[:, k, ti * 128:(ti + 1) * 128],
                            rhs=wp2[:, k, ng * 512:(ng + 1) * 512],
                            start=(k == 0), stop=(k == K8 - 1),
                        )
                        if last_dummy[0] is not None:
                            add_dep_helper(
                                mm.ins, last_dummy[0].ins, sync=False,
                                reason="proj after A2A-window warm-keeper",
                            )
                osb = stp.tile([128, D], F32, tag="osb", bufs=2)
                for ng in range(2):
                    nc.vector.tensor_add(
                        osb[:, ng * 512:(ng + 1) * 512], pspj[:, ng, :],
                        cv_b[:, ng * 512:(ng + 1) * 512],
                    )
                nc.sync.dma_start(
                    out=out[ti * 128:(ti + 1) * 128, :], in_=osb[:, :]
                )
    nc.compile()
    return nc


def _prep_inputs(x, w_atten, b_atten, w_proj, b_proj):
    x = np.asarray(x, dtype=np.float32)
    w_atten = np.asarray(w_atten, dtype=np.float32)
    b_atten = np.asarray(b_atten, dtype=np.float32)
    w_proj = np.asarray(w_proj, dtype=np.float32)
    b_proj = np.asarray(b_proj, dtype=np.float32)

    xT = np.ascontiguousarray(x.reshape(T, D).T).astype(NPBF16)
    wpT = np.ascontiguousarray(w_proj.T).astype(NPBF16)
    # v-bias routes through softmax as an additive constant: fold into cvec
    cvec = (b_atten[2 * D:3 * D] @ w_proj.T + b_proj).astype(np.float32)[None, :]

    in_maps = []
    for c in range(NC):
        r = slice(c * EC, (c + 1) * EC)
        wq = w_atten[0 * D:1 * D][r] * SCALE     # fold score scale into w_q
        wk = w_atten[1 * D:2 * D][r]
        wv = w_atten[2 * D:3 * D][r]
        wqkvT = np.ascontiguousarray(
            np.concatenate([wq.T, wk.T, wv.T], axis=1)
        ).astype(NPBF16)
        assert np.all(b_atten[:2 * D] == 0.0), "nonzero q/k bias unsupported"
        in_maps.append({
            "xT": xT, "wqkvT": wqkvT, "wpT": wpT,
            "cvec": cvec,
        })
    return in_maps


def _run(inputs: dict, trace: bool = False):
    if "nc" not in _CACHE:
        _CACHE["nc"] = _build_nc()
    nc = _CACHE["nc"]
    in_maps = _prep_inputs(**inputs)
    res = run_bass_kernel_spmd(nc, in_maps, core_ids=list(range(NC)), trace=trace)
    chunks = [res.results[c]["out"] for c in range(NC)]
    full = np.concatenate(chunks, axis=0).reshape(B, S, D).astype(np.float32)
    return full, res


def kernel(**inputs) -> np.ndarray:
    out, _ = _run(inputs, trace=False)
    return out


# revision 22
# speedup vs baseline: 1.0158x; 1.0158x over previous
"""Causal multi-head attention block on 8 TRN2 NeuronCores.

Sharding: tensor-parallel over heads (16 heads -> 2 per core) for QKV +
attention; AllToAll switches to token-parallel (4096 tokens -> 512 per
core) for the output projection.

Per-core device graph (all bf16 matmuls, fp32 PSUM accumulation):
  1. QKV: q^T,k^T in [e,t] layout (e = 2 heads x 64 on partitions),
     v in [t,e] layout, from a resident x^T [1024, 4096].
  2. Attention per (batch, head): scores computed TRANSPOSED
     s^T[kv, q] = k^T.T @ q^T so softmax stats land on the free axis of
     nothing -- instead the denominator comes free from a ones-column
     appended to v (rows of attn^T psum: 0:64 = numerator, 64 = denom).
     No max-subtraction (scores ~ N(0,1) after folding 1/sqrt(dk) into
     w_q on the host; exp can't overflow).
  3. Normalize via reciprocal + gpsimd partition-broadcast, stage to a
     DRAM AllToAll buffer as bf16.
  4. AllToAll (head-shards -> token-shards), then out = attn^T.T @ w_p^T
     per 512-token chunk, + host-folded constant bias vector.
"""

import numpy as np
import ml_dtypes

import concourse.bass as bass
import concourse.bacc as bacc
import concourse.mybir as mybir
from concourse.tile import TileContext, add_dep_helper
from concourse.bass_utils import run_bass_kernel_spmd

NC = 8                      # cores
B, S, D = 2, 2048, 1024
H, DK = 16, 64
HPC = H // NC               # heads per core = 2
EC = HPC * DK               # embed dims per core = 128
T = B * S                   # 4096 flattened tokens
TC = T // NC                # tokens per core chunk = 512
K8 = D // 128               # contraction tiles = 8
SCALE = 1.0 / np.sqrt(DK)

BF16 = mybir.dt.bfloat16
F32 = mybir.dt.float32
NPBF16 = ml_dtypes.bfloat16

_CACHE = {}


def _build_nc(dbg: bool = False) -> bass.Bass:
    nc = bacc.Bacc("TRN2", target_bir_lowering=False, debug=False, num_devices=NC)
    if dbg:
        dbg_q = nc.declare_dram_parameter("dbg_q", [EC, T], BF16, isOutput=True)
        dbg_k = nc.declare_dram_parameter("dbg_k", [EC, T], BF16, isOutput=True)
        dbg_v = nc.declare_dram_parameter("dbg_v", [128, T // 128 * 130], BF16, isOutput=True)
        dbg_ain = nc.declare_dram_parameter("dbg_ain", [NC, 128, 512], BF16, isOutput=True)
        dbg_aout = nc.declare_dram_parameter("dbg_aout", [NC, 128, 512], BF16, isOutput=True)

    xT = nc.declare_dram_parameter("xT", [D, T], BF16, isOutput=False)
    wqkvT = nc.declare_dram_parameter("wqkvT", [D, 3 * EC], BF16, isOutput=False)
    wpT = nc.declare_dram_parameter("wpT", [D, D], BF16, isOutput=False)
    cvec = nc.declare_dram_parameter("cvec", [1, D], F32, isOutput=False)
    out = nc.declare_dram_parameter("out", [TC, D], F32, isOutput=True)

    # causal masking happens on the PE: a matmul accumulates
    # ident.T @ maskneg = -60 on the strict lower triangle (kv > q) into
    # the diagonal score tiles, so exp() itself produces ~0 there and no
    # vector-engine op (and no extra ACT sync wait) is needed.
    ident_np = np.eye(128, dtype=NPBF16)
    maskneg_np = np.where(np.arange(128)[:, None] > np.arange(128)[None, :],
                          -60.0, 0.0).astype(NPBF16)
    ident_dram = nc.inline_tensor(ident_np, name="ident128")
    maskneg_dram = nc.inline_tensor(maskneg_np, name="maskneg128")

    with TileContext(nc) as tc:
        with (
            tc.tile_pool(name="const", bufs=1) as constp,
            tc.tile_pool(name="x", bufs=1) as xp,
            tc.tile_pool(name="qk", bufs=1) as qkp,
            tc.tile_pool(name="w", bufs=1) as wp,
            tc.tile_pool(name="ps", bufs=8, space="PSUM") as psp,
            tc.tile_pool(name="pt", bufs=6) as ptp,
            tc.tile_pool(name="nrm", bufs=2) as nrmp,
            tc.tile_pool(name="stage", bufs=4) as stp,
            tc.tile_pool(name="dram", bufs=1, space="DRAM") as dramp,
            tc.tile_pool(name="proj", bufs=1) as projp,
        ):
            # ---- constants ----
            # DMA-loaded tiles that feed DVE ops get "pre-touched" by a DVE
            # copy: the DVE clock absorbs the DMA-queue wait once, so the hot
            # consumers carry only their PE/ACT wait (the tensor_scalar ISA
            # struct can't encode two sync waits).
            ident_sb = constp.tile([128, 128], BF16)
            nc.sync.dma_start(out=ident_sb[:, :], in_=ident_dram[:, :])
            maskneg_sb = constp.tile([128, 128], BF16)
            nc.sync.dma_start(out=maskneg_sb[:, :], in_=maskneg_dram[:, :])
            cv_ld = constp.tile([128, D], F32)
            nc.gpsimd.dma_start(out=cv_ld[:, :], in_=cvec[:, :].to_broadcast([128, D]))
            cv_b = constp.tile([128, D], F32)
            nc.vector.tensor_copy(cv_b[:, :], cv_ld[:, :])

            # ---- load x^T and weights (interleaved so the first QKV
            # matmul's inputs land early; w_proj loads late) ----
            x_sb = xp.tile([128, K8, T], BF16)          # 8 MB
            wqkv_sb = wp.tile([128, K8, 3 * EC], BF16)
            for k in range(K8):
                nc.sync.dma_start(
                    out=wqkv_sb[:, k, :], in_=wqkvT[k * 128:(k + 1) * 128, :]
                )
                nc.sync.dma_start(out=x_sb[:, k, 0:T // 2],
                                  in_=xT[k * 128:(k + 1) * 128, 0:T // 2])
                nc.sync.dma_start(out=x_sb[:, k, T // 2:T],
                                  in_=xT[k * 128:(k + 1) * 128, T // 2:T])
            wp_sb = wp.tile([128, K8, D], BF16)         # w_proj^T (loaded late)

            q_sb = qkp.tile([EC, T], BF16)
            k_sb = qkp.tile([EC, T], BF16)
            # v layout: per 128-token tile, [v_h0(64) | ones | v_h1(64) | ones]
            v_sb = qkp.tile([128, T // 128, 130], BF16)
            nc.vector.memset(v_sb[:, :, 64:65], 1.0)    # ones column, head 0
            nc.vector.memset(v_sb[:, :, 129:130], 1.0)  # ones column, head 1

            # PSUM tag budget (8 banks): pso 3 + pss 2x2 + fill 1 = 8
            def qkv_stream(b):
                """Emit batch b's QKV, one instruction per next().

                q/k: two 512-token groups per 2-bank "pss" slot. v: four
                128-token tiles per slot sharing one bank -- only the very
                first matmul of the slot uses start=True (start clears the
                whole bank; zeroed regions accumulate correctly).
                """
                base = b * S
                for sec in (0, 1):
                    dst = q_sb if sec == 0 else k_sb
                    for np_ in range(2):                # pair of 512-groups
                        pq = psp.tile([128, 2, 512], F32, tag="pss", bufs=2,
                                      name=f"fq{b}{sec}{np_}")
                        for k in range(K8):
                            for n2 in range(2):
                                nc.tensor.matmul(
                                    pq[:, n2, :],
                                    lhsT=wqkv_sb[:, k, sec * EC:(sec + 1) * EC],
                                    rhs=x_sb[:, k, base + (np_ * 2 + n2) * 512:
                                             base + (np_ * 2 + n2 + 1) * 512],
                                    start=(k == 0), stop=(k == K8 - 1),
                                )
                                yield
                        nc.vector.tensor_copy(
                            dst[:, base + np_ * 1024:base + (np_ + 1) * 1024],
                            pq[:, :, :].rearrange("p a b -> p (a b)"),
                        )
                        yield
                for tq in range(4):                     # quad of v token-tiles
                    pv = psp.tile([128, 4, 128], F32, tag="pss", bufs=2,
                                  name=f"fv{b}{tq}")
                    for k in range(K8):
                        for t2 in range(4):
                            nc.tensor.matmul(
                                pv[:, t2, :],
                                lhsT=x_sb[:, k, base + (tq * 4 + t2) * 128:
                                          base + (tq * 4 + t2 + 1) * 128],
                                rhs=wqkv_sb[:, k, 2 * EC:3 * EC],
                                start=(k == 0 and t2 == 0), stop=(k == K8 - 1),
                            )
                            yield
                    src4 = pv[:, :, :].rearrange("p t (h e) -> p t h e", h=2)
                    dst4 = v_sb[:, b * 16 + tq * 4:b * 16 + (tq + 1) * 4, :]\
                        .rearrange("p t (h e) -> p t h e", e=65)[:, :, :, 0:64]
                    nc.vector.tensor_copy(dst4, src4)
                    yield

            last_dummy = [None]

            def dummy_stream():
                """Endless PE keep-warm matmuls into a scratch PSUM bank."""
                pd = psp.tile([128, 512], F32, tag="fill", bufs=1, name="dummy_ps")
                while True:
                    last_dummy[0] = nc.tensor.matmul(
                        pd[:, :], lhsT=wqkv_sb[:, 0, 0:EC],
                        rhs=x_sb[:, 0, 0:512], start=True, stop=True,
                    )
                    yield

            def drain(it, n):
                for _ in range(n):
                    try:
                        next(it)
                    except StopIteration:
                        return

            a2a_in = dramp.tile([NC, 128, 512], BF16, name="a2a_in")
            dummies = dummy_stream()

            def emit_attention(b, fill, fpi):
                """Attention for batch b; `fill` instructions are woven in at
                `fpi` per kv-tile iteration to keep the PE dense while ACT
                runs exp. Both heads share one 2-bank score tile so exp is a
                single ACT instruction per kv-tile."""
                gorder = range(S // 512) if b == 0 else \
                    reversed(range(S // 512))
                for g in gorder:                        # 4 query groups
                    pso = [psp.tile([128, 512], F32, tag="pso", bufs=3,
                                    name=f"pso_{b}_{g}_{hi}") for hi in range(2)]
                    nkv = 4 * (g + 1)
                    prev = None
                    for kj in range(nkv):
                        d = kj - 4 * g                  # >=0 on diagonal band
                        j0 = 128 * d if d >= 0 else 0
                        pss2 = psp.tile([128, 2, 512], F32, tag="pss", bufs=2,
                                        name=f"pss_{b}_{g}_{kj}")
                        for hi in range(2):
                            h0 = hi * DK
                            nc.tensor.matmul(
                                pss2[:, hi, j0:512],
                                lhsT=k_sb[h0:h0 + DK,
                                          b * S + kj * 128:b * S + (kj + 1) * 128],
                                rhs=q_sb[h0:h0 + DK,
                                         b * S + g * 512 + j0:b * S + (g + 1) * 512],
                                start=True, stop=(d < 0),
                            )
                            if d >= 0:
                                nc.tensor.matmul(
                                    pss2[:, hi, j0:j0 + 128],
                                    lhsT=ident_sb[:, :], rhs=maskneg_sb[:, :],
                                    start=False, stop=True,
                                )
                        pt2 = ptp.tile([128, 2, 512], BF16, tag="pt",
                                       name=f"pt_{b}_{g}_{kj}")
                        nc.scalar.activation(
                            pt2[:, :, j0:512], pss2[:, :, j0:512],
                            mybir.ActivationFunctionType.Exp,
                        )
                        if prev is not None:
                            ppt, pj0, pkj = prev
                            for hi in range(2):
                                nc.tensor.matmul(
                                    pso[hi][:65, pj0:512],
                                    lhsT=v_sb[:, (b * S) // 128 + pkj,
                                              hi * 65:(hi + 1) * 65],
                                    rhs=ppt[:, hi, pj0:512],
                                    start=(pkj == 0), stop=False,
                                )
                        drain(fill, fpi)
                        prev = (pt2, j0, kj)
                    ppt, pj0, pkj = prev
                    for hi in range(2):
                        nc.tensor.matmul(
                            pso[hi][:65, pj0:512],
                            lhsT=v_sb[:, (b * S) // 128 + pkj,
                                      hi * 65:(hi + 1) * 65],
                            rhs=ppt[:, hi, pj0:512],
                            start=(pkj == 0), stop=True,
                        )
                    # normalize rows 0:64 by row 64, stage for A2A
                    for hi in range(2):
                        dn = nrmp.tile([1, 512], F32, tag="dn")
                        nc.vector.tensor_copy(dn[:, :], pso[hi][64:65, :])
                        ddr = dramp.tile([1, 512], F32, tag="ddr", bufs=4,
                                         name="ddr")
                        nc.gpsimd.dma_start(out=ddr[:, :], in_=dn[:, :])
                        dnb = nrmp.tile([64, 512], F32, tag="dnb")
                        nc.gpsimd.dma_start(
                            out=dnb[:, :], in_=ddr[:, :].to_broadcast([64, 512])
                        )
                        rb = nrmp.tile([64, 512], F32, tag="rb")
                        nc.vector.reciprocal_approx_fast(rb[:, :], dnb[:, :])
                        aout = stp.tile([64, 512], BF16, tag="aout", bufs=3)
                        nc.vector.tensor_mul(aout[:, :], pso[hi][0:64, :], rb[:, :])
                        chunk = b * (S // 512) + g
                        nc.sync.dma_start(
                            out=a2a_in[chunk, hi * 64:(hi + 1) * 64, :],
                            in_=aout[:, :],
                        )

            # batch 0 QKV runs dense; batch 1 QKV weaves into batch 0's
            # attention; dummies keep HAM warm through batch 1's attention.
            qkv0 = qkv_stream(0)
            drain(qkv0, 10 ** 6)
            fill1 = qkv_stream(1)
            emit_attention(0, fill1, 3)
            drain(fill1, 10 ** 6)
            emit_attention(1, dummies, 1)

            if dbg:
                nc.sync.dma_start(out=dbg_q[:, :], in_=q_sb[:, :])
                nc.sync.dma_start(out=dbg_k[:, :], in_=k_sb[:, :])
                nc.sync.dma_start(out=dbg_v[:, :], in_=v_sb[:, :, :].rearrange("p a b -> p (a b)"))
                nc.sync.dma_start(out=dbg_ain[:, :, :], in_=a2a_in[:, :, :])
            # w_proj loads during attention; A2A bridge with PE kept warm
            for k in range(K8):
                nc.sync.dma_start(out=wp_sb[:, k, :], in_=wpT[k * 128:(k + 1) * 128, :])
            a2a_out = dramp.tile([NC, 128, 512], BF16, name="a2a_out")
            nc.gpsimd.collective_compute(
                "AllToAll",
                mybir.AluOpType.bypass,
                ins=[a2a_in.opt()],
                outs=[a2a_out.opt()],
                replica_groups=[list(range(NC))],
            )
            drain(dummies, 128)

            if dbg:
                nc.sync.dma_start(out=dbg_aout[:, :, :], in_=a2a_out[:, :, :])
            # ---- output projection on my 512-token chunk ----
            at_sb = projp.tile([128, NC, 512], BF16)
            for k in range(NC):
                nc.sync.dma_start(out=at_sb[:, k, :], in_=a2a_out[k, :, :])
            # redirect proj reads through DVE copies: the matmuls then wait
            # only on the DVE semaphore (one wait -> no LDWEIGHTS hoisting,
            # which otherwise stalls the PE queue on the A2A)
            at2 = projp.tile([128, NC, 512], BF16)
            nc.vector.tensor_copy(at2[:, :, :], at_sb[:, :, :])
            wp2 = projp.tile([128, K8, D], BF16)
            nc.vector.tensor_copy(wp2[:, :, :], wp_sb[:, :, :])
            for ti in range(TC // 128):
                # "pso"-tag psum: its releases are DVE-side (aout mul), so
                # proj matmuls carry a single DVE wait -> no LDW hoisting
                pspj = [psp.tile([128, 512], F32, tag="pso", bufs=3,
                                 name=f"pspj_{ti}_{ng}") for ng in range(2)]
                for ng in range(2):
                    for k in range(K8):
                        mm = nc.tensor.matmul(
                            pspj[ng][:, :],
                            lhsT=at2[:, k, ti * 128:(ti + 1) * 128],
                            rhs=wp2[:, k, ng * 512:(ng + 1) * 512],
                            start=(k == 0), stop=(k == K8 - 1),
                        )
                        if last_dummy[0] is not None:
                            add_dep_helper(
                                mm.ins, last_dummy[0].ins, sync=False,
                                reason="proj after A2A-window warm-keeper",
                            )
                osb = stp.tile([128, D], F32, tag="osb", bufs=2)
                for ng in range(2):
                    nc.vector.tensor_add(
                        osb[:, ng * 512:(ng + 1) * 512], pspj[ng][:, :],
                        cv_b[:, ng * 512:(ng + 1) * 512],
                    )
                nc.sync.dma_start(
                    out=out[ti * 128:(ti + 1) * 128, :], in_=osb[:, :]
                )
    nc.compile()
    return nc


def _prep_inputs(x, w_atten, b_atten, w_proj, b_proj):
    x = np.asarray(x, dtype=np.float32)
    w_atten = np.asarray(w_atten, dtype=np.float32)
    b_atten = np.asarray(b_atten, dtype=np.float32)
    w_proj = np.asarray(w_proj, dtype=np.float32)
    b_proj = np.asarray(b_proj, dtype=np.float32)

    xT = np.ascontiguousarray(x.reshape(T, D).T).astype(NPBF16)
    wpT = np.ascontiguousarray(w_proj.T).astype(NPBF16)
    # v-bias routes through softmax as an additive constant: fold into cvec
    cvec = (b_atten[2 * D:3 * D] @ w_proj.T + b_proj).astype(np.float32)[None, :]

    in_maps = []
    for c in range(NC):
        r = slice(c * EC, (c + 1) * EC)
        wq = w_atten[0 * D:1 * D][r] * SCALE     # fold score scale into w_q
        wk = w_atten[1 * D:2 * D][r]
        wv = w_atten[2 * D:3 * D][r]
        wqkvT = np.ascontiguousarray(
            np.concatenate([wq.T, wk.T, wv.T], axis=1)
        ).astype(NPBF16)
        assert np.all(b_atten[:2 * D] == 0.0), "nonzero q/k bias unsupported"
        in_maps.append({
            "xT": xT, "wqkvT": wqkvT, "wpT": wpT,
            "cvec": cvec,
        })
    return in_maps


def _run(inputs: dict, trace: bool = False):
    if "nc" not in _CACHE:
        _CACHE["nc"] = _build_nc()
    nc = _CACHE["nc"]
    in_maps = _prep_inputs(**inputs)
    res = run_bass_kernel_spmd(nc, in_maps, core_ids=list(range(NC)), trace=trace)
    chunks = [res.results[c]["out"] for c in range(NC)]
    full = np.concatenate(chunks, axis=0).reshape(B, S, D).astype(np.float32)
    return full, res


def kernel(**inputs) -> np.ndarray:
    out, _ = _run(inputs, trace=False)
    return out
